# revision 8
# baseline (speedup 1.0000x reference)
"""Trainium2 Bass kernel for the ANI (anisotropy) L1 loss.

Math (per voxel, per 3x3 symmetric tensor with channels xx,xy,xz,yy,yz,zz):
  y_c = gt_std[c] * x_c + gt_mean[c]            (affine; mask applied at the end)
  A   = [[y0,y1,y2],[y1,y3,y4],[y2,y4,y5]]
  q   = tr(A)/3 ;  C = A - q I
  p2  = ||C||_F^2 ;  p = sqrt(p2/6) ;  det = det(C)
  r   = det / (2 p^3) in [-1,1] ;  phi = arccos(r)/3
  ani(input)  = 3 p cos(phi)          (= l2 - (l0+l1)/2)
  ani(target) = q - p cos(phi)        (= (l0+l1)/2)
  loss = sum(|ani_in - ani_tg| * m) / max(sum(m), 1)

On-device identities (HW tables lack arccos/cos/rsqrt):
  cos(arccos(r)/3) = sin(pi/3 + arctan(w)/3)
  w = r/sqrt(1-r^2) = sqrt(54) * det / sqrt(g),  g = p2^3 - 54 det^2
(no normalization by p^3 ever happens; g is computed by one fused custom
DVE op with a floor clamp, then 1/g via RECIPROCAL_APPROX_FAST and sqrt on ACT).

Sharding: pure data-parallel over the flattened spatial axis (8 cores); each
core returns per-partition (masked |diff| sum, mask count) partials and the
host reduces them to the scalar loss.
"""

import numpy as np

import concourse.bass as bass
import concourse.tile as tile
from concourse import bacc, mybir
from concourse.bass_utils import run_bass_kernel_spmd

F32 = mybir.dt.float32
BF16 = mybir.dt.bfloat16
I32 = mybir.dt.int32
ALU = mybir.AluOpType
AF = mybir.ActivationFunctionType

N_CORES = 8
B, C = 4, 6
HWD = 96 * 96 * 96          # 884736
SH = HWD // N_CORES         # 110592
BSH = B * SH                # 442368 voxels per core
P = 128
FREE = BSH // P             # 3456
NT = 1728                   # free elems per tile
NCH = FREE // NT            # chunks

SQRT54 = float(np.sqrt(54.0))
GMIN = 1e-30
PEPS = 1e-25
PI3 = float(np.pi / 3.0)

_CACHE = {}

# bisect/config flags
USE_CUSTOM_GCLAMP = True
USE_FAST_RECIP = True
MASK_FUSED = True          # mask convert+count in one tensor_scalar(accum_out)
AFFINE_ON_ACT = True       # per-channel affine on ScalarE instead of DVE

# ---------------------------------------------------------------------------
# Custom fused DVE op:  gc = max(p2^3 - 54*det^2, GMIN)
# ---------------------------------------------------------------------------
_GCLAMP = None


def _register_gclamp():
    global _GCLAMP
    if _GCLAMP is not None:
        return _GCLAMP
    import concourse.dve_ops as dve_ops
    from concourse.dve_ops import DveOp
    from concourse.dve_spec import Spec, Src0, Src1, C0, C2, maxx, sq, lower, _has_src1
    from concourse.dve_uop import DveOpSpec

    name = "ANI_GCLAMP"
    body = maxx((sq(Src0) * Src0) - (sq(Src1) * C2), C0)

    def ref(in0, in1, c0, c1, c2):
        x = in0.astype(np.float32)
        d = in1.astype(np.float32)
        return np.maximum(x * x * x - d * d * c2, c0)

    spec = Spec(body=body, reference=ref)
    row = dve_ops._CUSTOM_DVE_ROW_BASE + len(dve_ops.OPS)
    tmp = DveOpSpec(name=name, opcode=row, uops=lower(spec, ver="v3"),
                    rd1_en=_has_src1(spec))
    op = DveOp(name, spec, subdim=False, uops_sha={"v3": tmp.sha("v3")})
    dve_ops.OPS.append(op)
    dve_ops.CUSTOM_DVE_SPECS[name] = spec
    dve_ops._SUB_OPCODE_FOR_NAME[name] = row
    _GCLAMP = op
    return op


def _build(reps: int = 1):
    gclamp = _register_gclamp() if USE_CUSTOM_GCLAMP else None
    nc = bacc.Bacc("TRN2", target_bir_lowering=False, debug=False,
                   num_devices=N_CORES)
    x_in = nc.dram_tensor("input_data", [C, BSH], F32, kind="ExternalInput")
    t_in = nc.dram_tensor("target", [C, BSH], F32, kind="ExternalInput")
    m_in = nc.dram_tensor("mask", [BSH], I32, kind="ExternalInput")
    sc_in = nc.dram_tensor("scal", [P, 16], F32, kind="ExternalInput")
    out = nc.dram_tensor("out", [P, 2], F32, kind="ExternalOutput")

    with tile.TileContext(nc) as tc:
        with (
            tc.tile_pool(name="const", bufs=1) as cpool,
            tc.tile_pool(name="io", bufs=2) as iopool,
            tc.tile_pool(name="tmp", bufs=1) as tpool,
            tc.tile_pool(name="acc", bufs=1) as apool,
            tc.tile_pool(name="part", bufs=2) as ppool,
        ):
            scal = cpool.tile([P, 16], F32, tag="scal")
            nc.sync.dma_start(scal[:], sc_in[:])
            lacc = apool.tile([P, 1], F32, tag="lacc")
            cacc = apool.tile([P, 1], F32, tag="cacc")
            nc.vector.memset(lacc[:], 0.0)
            nc.vector.memset(cacc[:], 0.0)

            def s_ap(c):
                return scal[:, c:c + 1]

            def mu_ap(c):
                return scal[:, 6 + c:7 + c]

            peps_ap = scal[:, 12:13]
            pi3_ap = scal[:, 13:14]

            def tensor_chain(src, off, name):
                """Compute (p2, det, rec=1/gc, q) tiles for one tensor.
                Scratch (y*, s*) tags are shared between input/target phases."""
                xs = []
                for c in range(C):
                    xt = iopool.tile([P, NT], F32, tag=f"x{c}")
                    nc.sync.dma_start(
                        xt[:],
                        src[c].rearrange("(p f) -> p f", p=P)[:, off:off + NT])
                    xs.append(xt)
                # y tiles (bf16); c00/c11/s01 overwrite y0/y3/y5 in place
                y = []
                for c in range(C):
                    yt = tpool.tile([P, NT], BF16, tag=f"y{c}")
                    if AFFINE_ON_ACT:
                        nc.scalar.activation(yt[:], xs[c][:], AF.Identity,
                                             bias=mu_ap(c), scale=s_ap(c))
                    else:
                        nc.vector.tensor_scalar(
                            yt[:], xs[c][:], s_ap(c), mu_ap(c), ALU.mult, ALU.add)
                    y.append(yt)

                s1 = tpool.tile([P, NT], BF16, tag="s1")   # tr
                nc.vector.tensor_tensor(s1[:], y[0][:], y[3][:], ALU.add)
                s2 = tpool.tile([P, NT], BF16, tag="s2")   # tr2
                nc.vector.tensor_tensor(s2[:], s1[:], y[5][:], ALU.add)
                q = tpool.tile([P, NT], BF16, tag=f"q{name}")
                nc.vector.tensor_scalar(q[:], s2[:], 1.0 / 3.0, None, ALU.mult)

                c00 = y[0]
                nc.vector.tensor_tensor(c00[:], y[0][:], q[:], ALU.subtract)
                c11 = y[3]
                nc.vector.tensor_tensor(c11[:], y[3][:], q[:], ALU.subtract)
                s01 = y[5]                                  # s01 = -c22 = q - y5
                nc.vector.tensor_tensor(s01[:], q[:], y[5][:], ALU.subtract)

                # squares (ACT; Square lives in every table set)
                sq00 = tpool.tile([P, NT], BF16, tag="s3")
                nc.scalar.activation(sq00[:], c00[:], AF.Square)
                sq11 = tpool.tile([P, NT], BF16, tag="s4")
                nc.scalar.activation(sq11[:], c11[:], AF.Square)
                sq22 = tpool.tile([P, NT], BF16, tag="s5")
                nc.scalar.activation(sq22[:], s01[:], AF.Square)
                o1 = tpool.tile([P, NT], BF16, tag="s6")
                nc.scalar.activation(o1[:], y[1][:], AF.Square)
                o2 = tpool.tile([P, NT], BF16, tag="s7")
                nc.scalar.activation(o2[:], y[2][:], AF.Square)
                o3 = tpool.tile([P, NT], BF16, tag="s8")
                nc.scalar.activation(o3[:], y[4][:], AF.Square)

                # p2 = sq00+sq11+sq22 + 2*(o1+o2+o3); chains in place
                nc.vector.tensor_tensor(sq00[:], sq00[:], sq11[:], ALU.add)
                nc.vector.tensor_tensor(sq00[:], sq00[:], sq22[:], ALU.add)
                nc.vector.tensor_tensor(s2[:], o1[:], o2[:], ALU.add)
                nc.vector.tensor_tensor(s2[:], s2[:], o3[:], ALU.add)
                nc.vector.tensor_scalar(s2[:], s2[:], 2.0, None, ALU.mult)
                p2 = tpool.tile([P, NT], BF16, tag=f"p2{name}")
                nc.vector.tensor_tensor(p2[:], sq00[:], s2[:], ALU.add)

                # det = s01*(o1 - c00*c11) - (c00*o3 + c11*o2) + 2*y1*y2*y4
                nc.vector.tensor_tensor(sq11[:], c00[:], c11[:], ALU.mult)       # Pm
                nc.vector.tensor_tensor(sq11[:], o1[:], sq11[:], ALU.subtract)   # K
                nc.vector.tensor_tensor(sq11[:], s01[:], sq11[:], ALU.mult)      # T1
                nc.vector.tensor_tensor(o3[:], c00[:], o3[:], ALU.mult)          # A_
                nc.vector.tensor_tensor(o2[:], c11[:], o2[:], ALU.mult)          # B_
                nc.vector.tensor_tensor(o3[:], o3[:], o2[:], ALU.add)            # S_
                nc.vector.tensor_tensor(sq11[:], sq11[:], o3[:], ALU.subtract)   # D_
                nc.vector.tensor_tensor(y[1][:], y[1][:], y[2][:], ALU.mult)     # Y
                nc.vector.tensor_tensor(y[1][:], y[1][:], y[4][:], ALU.mult)     # Y2
                nc.vector.tensor_scalar(y[1][:], y[1][:], 2.0, None, ALU.mult)
                det = tpool.tile([P, NT], BF16, tag=f"det{name}")
                nc.vector.tensor_tensor(det[:], sq11[:], y[1][:], ALU.add)

                # gc = max(p2^3 - 54 det^2, GMIN) fused; rec = 1/gc
                gc = tpool.tile([P, NT], F32, tag=f"gc{name}")
                if USE_CUSTOM_GCLAMP:
                    nc.vector._custom_dve(gclamp, out=gc[:], in0=p2[:],
                                          in1=det[:], s0=GMIN, imm2=54.0)
                else:
                    p2sq = tpool.tile([P, NT], F32, tag="p2sq")
                    nc.scalar.activation(p2sq[:], p2[:], AF.Square)
                    nc.vector.tensor_tensor(p2sq[:], p2sq[:], p2[:], ALU.mult)
                    dsq = tpool.tile([P, NT], F32, tag="dsq")
                    nc.scalar.activation(dsq[:], det[:], AF.Square, scale=SQRT54)
                    nc.vector.tensor_tensor(gc[:], p2sq[:], dsq[:], ALU.subtract)
                    nc.vector.tensor_scalar(gc[:], gc[:], GMIN, None, ALU.max)
                if USE_FAST_RECIP:
                    nc.vector.reciprocal_approx_fast(gc[:], gc[:])
                else:
                    rec2 = tpool.tile([P, NT], F32, tag=f"rec{name}")
                    nc.vector.reciprocal(rec2[:], gc[:])
                    gc = rec2
                return {"p2": p2, "det": det, "rec": gc, "q": q}

            for _ in range(reps):
                for ch in range(NCH):
                    off = ch * NT
                    mt = iopool.tile([P, NT], I32, tag="mask")
                    nc.sync.dma_start(
                        mt[:],
                        m_in.rearrange("(p f) -> p f", p=P)[:, off:off + NT])
                    mf = tpool.tile([P, NT], BF16, tag="mf")
                    if MASK_FUSED:
                        msum = ppool.tile([P, 1], F32, tag="msum")
                        nc.scalar.activation(mf[:], mt[:], AF.Copy,
                                             accum_out=msum[:])
                        nc.vector.tensor_tensor(cacc[:], cacc[:], msum[:],
                                                ALU.add)
                    else:
                        nc.vector.tensor_copy(mf[:], mt[:])

                    sti = tensor_chain(x_in, off, "i")
                    stt = tensor_chain(t_in, off, "t")

                    # ---- sqrt-set batch ----
                    p3 = sti["p2"]          # in place: p3 = sqrt(1.5*p2+eps)
                    nc.scalar.activation(p3[:], sti["p2"][:], AF.Sqrt,
                                         bias=peps_ap, scale=1.5)
                    p1 = stt["p2"]
                    nc.scalar.activation(p1[:], stt["p2"][:], AF.Sqrt,
                                         bias=peps_ap, scale=1.0 / 6.0)
                    rsgi = tpool.tile([P, NT], BF16, tag="s1")
                    nc.scalar.activation(rsgi[:], sti["rec"][:], AF.Sqrt)
                    rsgt = tpool.tile([P, NT], BF16, tag="s2")
                    nc.scalar.activation(rsgt[:], stt["rec"][:], AF.Sqrt)

                    # ---- wk = det * rsqrt(g) (DVE) ----
                    wki = sti["det"]
                    nc.vector.tensor_tensor(wki[:], sti["det"][:], rsgi[:],
                                            ALU.mult)
                    wkt = stt["det"]
                    nc.vector.tensor_tensor(wkt[:], stt["det"][:], rsgt[:],
                                            ALU.mult)

                    # ---- trig-set batch ----
                    nc.scalar.activation(wki[:], wki[:], AF.Arctan, scale=SQRT54)
                    nc.scalar.activation(wki[:], wki[:], AF.Sin,
                                         bias=pi3_ap, scale=1.0 / 3.0)  # cs_i
                    nc.scalar.activation(wkt[:], wkt[:], AF.Arctan, scale=SQRT54)
                    nc.scalar.activation(wkt[:], wkt[:], AF.Sin,
                                         bias=pi3_ap, scale=1.0 / 3.0)  # cs_t

                    # ---- final: |3 p_i cs_i + p_t cs_t - q_t| * m ----
                    nc.vector.tensor_tensor(wki[:], p3[:], wki[:], ALU.mult)   # u
                    nc.vector.tensor_tensor(wkt[:], p1[:], wkt[:], ALU.mult)   # v
                    nc.vector.tensor_tensor(wki[:], wki[:], wkt[:], ALU.add)   # w3
                    nc.vector.tensor_tensor(wki[:], wki[:], stt["q"][:],
                                            ALU.subtract)                      # diff
                    nc.vector.tensor_tensor(wki[:], wki[:], mf[:], ALU.mult)   # dm

                    asum = ppool.tile([P, 1], F32, tag="asum")
                    nc.scalar.activation(wki[:], wki[:], AF.Abs,
                                         accum_out=asum[:])
                    nc.vector.tensor_tensor(lacc[:], lacc[:], asum[:], ALU.add)
                    if not MASK_FUSED:
                        msum = ppool.tile([P, 1], F32, tag="msum")
                        nc.vector.tensor_reduce(msum[:], mf[:],
                                                mybir.AxisListType.X, ALU.add)
                        nc.vector.tensor_tensor(cacc[:], cacc[:], msum[:],
                                                ALU.add)

            res = apool.tile([P, 2], F32, tag="res")
            nc.vector.tensor_copy(res[:, 0:1], lacc[:])
            nc.vector.tensor_copy(res[:, 1:2], cacc[:])
            nc.sync.dma_start(out[:], res[:])

    nc.compile()
    return nc


def get_module(reps: int = 1):
    if reps not in _CACHE:
        _CACHE[reps] = _build(reps)
    return _CACHE[reps]


def make_in_maps(input_data, target, mask, gt_mean, gt_std):
    xs = np.asarray(input_data).reshape(B, C, HWD)
    ts = np.asarray(target).reshape(B, C, HWD)
    ms = np.asarray(mask).reshape(B, HWD)
    scal = np.zeros((P, 16), np.float32)
    scal[:, 0:6] = np.asarray(gt_std, np.float32).reshape(1, 6)
    scal[:, 6:12] = np.asarray(gt_mean, np.float32).reshape(1, 6)
    scal[:, 12] = PEPS
    scal[:, 13] = PI3
    in_maps = []
    for k in range(N_CORES):
        sl = slice(k * SH, (k + 1) * SH)
        in_maps.append({
            "input_data": np.ascontiguousarray(
                xs[:, :, sl].transpose(1, 0, 2)).reshape(C, BSH),
            "target": np.ascontiguousarray(
                ts[:, :, sl].transpose(1, 0, 2)).reshape(C, BSH),
            "mask": np.ascontiguousarray(ms[:, sl]).reshape(BSH),
            "scal": scal,
        })
    return in_maps


def kernel(input_data, target, mask, gt_mean, gt_std):
    nc = get_module()
    in_maps = make_in_maps(input_data, target, mask, gt_mean, gt_std)
    r = run_bass_kernel_spmd(nc, in_maps, core_ids=list(range(N_CORES)))
    s = 0.0
    n = 0.0
    for i in range(N_CORES):
        o = r.results[i]["out"].astype(np.float64)
        s += o[:, 0].sum()
        n += o[:, 1].sum()
    return np.float32(s / max(n, 1.0))


# revision 9
# speedup vs baseline: 1.0154x; 1.0154x over previous
"""Trainium2 Bass kernel for the ANI (anisotropy) L1 loss - final version.

Math (per voxel, 3x3 symmetric tensor, channels xx,xy,xz,yy,yz,zz):
  y_c = gt_std[c]*x_c + gt_mean[c]
  A = [[y0,y1,y2],[y1,y3,y4],[y2,y4,y5]];  q = tr(A)/3;  C = A - q I
  p2 = ||C||_F^2; p = sqrt(p2/6); det = det(C); r = det/(2 p^3)
  phi = arccos(r)/3
  ani_in  = 3 p cos(phi)        ani_tg = q - p cos(phi)
  loss = sum(|ani_in - ani_tg| * mask) / max(sum(mask), 1)

Device identities (ACT tables lack arccos/cos/rsqrt):
  cos(arccos(r)/3) = sin(pi/3 + arctan(w)/3),  w = r/sqrt(1-r^2)
  w = sqrt(6.75) * det / sqrt(gc),  gc = max(e^3 - 6.75 det^2, GMIN)
  where e = p2/2 (the traceless identity sq00+sq11+sq22 = 2(sq00+sq11+n00*n11)
  removes one square; the factor 2 is folded into constant scales).
  3p_in = sqrt(3 e + eps);  p_tg = sqrt(e/3 + eps).

Mapping: bf16 mid-chain on DVE (tensor ops never write in place - that would
break the 2x bf16 perf mode), affines/squares/sqrt/arctan/sin/abs on ScalarE
(Square/Abs live in every ACT table set; the two Sqrt ops are emitted
adjacently per chain to minimize table-set switches), one fused custom DVE op
for gc, RECIPROCAL_APPROX_FAST for 1/gc. Masked |diff| and mask count reduce
to [128,1] partials via accum_out.

Sharding: pure data-parallel, spatial axis split 8 ways. Each core emits
[128,2] (masked-|diff| sum, mask count) partials; the host reduces the 8x128
pairs and divides - that is the "all-reduce of (masked-sum, mask-count)".
"""

import numpy as np

import concourse.tile as tile
from concourse import bacc, mybir
from concourse.bass_utils import run_bass_kernel_spmd

F32 = mybir.dt.float32
BF16 = mybir.dt.bfloat16
I32 = mybir.dt.int32
ALU = mybir.AluOpType
AF = mybir.ActivationFunctionType

N_CORES = 8
B, C = 4, 6
HWD = 96 * 96 * 96
SH = HWD // N_CORES         # spatial elems per core per (b, c)
BSH = B * SH                # 442368 voxels per core
P = 128
FREE = BSH // P             # 3456
NT = 1728                   # free elems per chunk (2 chunks)
XBUF = 3

SQRT675 = float(np.sqrt(6.75))
GMIN = 1e-30
PEPS = 1e-25
PI3 = float(np.pi / 3.0)

_CACHE = {}
_GCLAMP = None


def _register_gclamp():
    """Register the fused custom DVE op gc = max(in0^3 - imm2*in1^2, s0)."""
    global _GCLAMP
    if _GCLAMP is not None:
        return _GCLAMP
    import concourse.dve_ops as dve_ops
    from concourse.dve_ops import DveOp
    from concourse.dve_spec import Spec, Src0, Src1, C0, C2, maxx, sq, lower, _has_src1
    from concourse.dve_uop import DveOpSpec

    name = "ANI_GCLAMP"
    body = maxx((sq(Src0) * Src0) - (sq(Src1) * C2), C0)

    def ref(in0, in1, c0, c1, c2):
        x = in0.astype(np.float32)
        d = in1.astype(np.float32)
        return np.maximum(x * x * x - d * d * c2, c0)

    spec = Spec(body=body, reference=ref)
    row = dve_ops._CUSTOM_DVE_ROW_BASE + len(dve_ops.OPS)
    tmp = DveOpSpec(name=name, opcode=row, uops=lower(spec, ver="v3"),
                    rd1_en=_has_src1(spec))
    op = DveOp(name, spec, subdim=False, uops_sha={"v3": tmp.sha("v3")})
    dve_ops.OPS.append(op)
    dve_ops.CUSTOM_DVE_SPECS[name] = spec
    dve_ops._SUB_OPCODE_FOR_NAME[name] = row
    _GCLAMP = op
    return op


def _build(reps: int = 1):
    NCH = FREE // NT
    gclamp = _register_gclamp()
    nc = bacc.Bacc("TRN2", target_bir_lowering=False, debug=False,
                   num_devices=N_CORES)
    x_in = nc.dram_tensor("input_data", [C, BSH], F32, kind="ExternalInput")
    t_in = nc.dram_tensor("target", [C, BSH], F32, kind="ExternalInput")
    m_in = nc.dram_tensor("mask", [BSH], I32, kind="ExternalInput")
    sc_in = nc.dram_tensor("scal", [P, 16], F32, kind="ExternalInput")
    out = nc.dram_tensor("out", [P, 2], F32, kind="ExternalOutput")

    with tile.TileContext(nc) as tc:
        with (
            tc.tile_pool(name="const", bufs=1) as cpool,
            tc.tile_pool(name="xio", bufs=XBUF) as xpool,
            tc.tile_pool(name="mio", bufs=1) as mpool,
            tc.tile_pool(name="tmp", bufs=1) as tpool,
            tc.tile_pool(name="acc", bufs=1) as apool,
            tc.tile_pool(name="part", bufs=2) as ppool,
        ):
            scal = cpool.tile([P, 16], F32, tag="scal")
            nc.sync.dma_start(scal[:], sc_in[:])
            lacc = apool.tile([P, 1], F32, tag="lacc")
            cacc = apool.tile([P, 1], F32, tag="cacc")
            nc.vector.memset(lacc[:], 0.0)
            nc.vector.memset(cacc[:], 0.0)

            def s_ap(c):
                return scal[:, c:c + 1]

            def mu_ap(c):
                return scal[:, 6 + c:7 + c]

            peps_ap = scal[:, 12:13]
            pi3_ap = scal[:, 13:14]

            _cnt = [0]

            def bt(tag):
                _cnt[0] += 1
                return tpool.tile([P, NT], BF16, tag=tag,
                                  name=f"b{tag}_{_cnt[0]}")

            def ft(tag):
                _cnt[0] += 1
                return tpool.tile([P, NT], F32, tag=tag,
                                  name=f"f{tag}_{_cnt[0]}")

            def chain(src, off, nm, p_scale):
                """One tensor's full chain. Tag namespace `nm` keeps the two
                tensors' chains independent so the scheduler interleaves them.
                Pool slots are recycled across stages (comments note the dead
                tile being replaced); no DVE op ever writes its own input.
                Returns (p, cs, tr2): p = sqrt(p_scale*e+eps), cs = cos term.
                """
                y = []
                for c in range(C):
                    _cnt[0] += 1
                    xt = xpool.tile([P, NT], F32, tag="x",
                                    name=f"x{nm}{c}_{_cnt[0]}")
                    nc.sync.dma_start(
                        xt[:],
                        src[c].rearrange("(p f) -> p f", p=P)[:, off:off + NT])
                    yt = bt(f"{nm}y{c}")
                    nc.scalar.activation(yt[:], xt[:], AF.Identity,
                                         bias=mu_ap(c), scale=s_ap(c))
                    y.append(yt)

                tr = bt(f"{nm}sa")
                nc.vector.tensor_tensor(tr[:], y[0][:], y[3][:], ALU.add)
                tr2 = bt(f"{nm}tr2")
                nc.vector.tensor_tensor(tr2[:], tr[:], y[5][:], ALU.add)
                q = bt(f"{nm}q")
                nc.vector.tensor_scalar(q[:], tr2[:], 1.0 / 3.0, None, ALU.mult)
                n00 = bt(f"{nm}n00")     # negated deviator diag: n = q - y
                nc.vector.tensor_tensor(n00[:], q[:], y[0][:], ALU.subtract)
                n11 = bt(f"{nm}n11")
                nc.vector.tensor_tensor(n11[:], q[:], y[3][:], ALU.subtract)
                n22 = bt(f"{nm}n22")
                nc.vector.tensor_tensor(n22[:], q[:], y[5][:], ALU.subtract)

                sq00 = bt(f"{nm}y0")     # y0 dead
                nc.scalar.activation(sq00[:], n00[:], AF.Square)
                sq11 = bt(f"{nm}y3")     # y3 dead
                nc.scalar.activation(sq11[:], n11[:], AF.Square)
                o1 = bt(f"{nm}o1")
                nc.scalar.activation(o1[:], y[1][:], AF.Square)
                o2 = bt(f"{nm}o2")
                nc.scalar.activation(o2[:], y[2][:], AF.Square)
                o3 = bt(f"{nm}o3")
                nc.scalar.activation(o3[:], y[4][:], AF.Square)

                pm = bt(f"{nm}q")        # q dead
                nc.vector.tensor_tensor(pm[:], n00[:], n11[:], ALU.mult)

                # e = p2/2 = (sq00+sq11+pm) + (o1+o2+o3), clamped >= 0
                a1 = bt(f"{nm}sa")       # tr dead
                nc.vector.tensor_tensor(a1[:], sq00[:], sq11[:], ALU.add)
                a2 = bt(f"{nm}y5")       # y5 dead
                nc.vector.tensor_tensor(a2[:], a1[:], pm[:], ALU.add)
                b1 = bt(f"{nm}b1")
                nc.vector.tensor_tensor(b1[:], o1[:], o2[:], ALU.add)
                b2 = bt(f"{nm}sa")       # a1 dead
                nc.vector.tensor_tensor(b2[:], b1[:], o3[:], ALU.add)
                e0 = bt(f"{nm}p0")
                nc.vector.tensor_tensor(e0[:], a2[:], b2[:], ALU.add)
                e = bt(f"{nm}p2")        # bf16 rounding can leave e0 < 0 in
                nc.vector.tensor_scalar(  # near-isotropic voxels; sqrt needs >=0
                    e[:], e0[:], 0.0, None, ALU.max)

                # det = n22*(o1 - pm) + n00*o3 + n11*o2 + 2*y1*y2*y4
                K = bt(f"{nm}b1")        # b1 dead
                nc.vector.tensor_tensor(K[:], o1[:], pm[:], ALU.subtract)
                T1 = bt(f"{nm}sa")       # b2 dead
                nc.vector.tensor_tensor(T1[:], n22[:], K[:], ALU.mult)
                A_ = bt(f"{nm}y0")       # sq00 dead
                nc.vector.tensor_tensor(A_[:], n00[:], o3[:], ALU.mult)
                B_ = bt(f"{nm}y3")       # sq11 dead
                nc.vector.tensor_tensor(B_[:], n11[:], o2[:], ALU.mult)
                S_ = bt(f"{nm}o2")       # o2 dead
                nc.vector.tensor_tensor(S_[:], A_[:], B_[:], ALU.add)
                D_ = bt(f"{nm}y5")       # a2 dead
                nc.vector.tensor_tensor(D_[:], T1[:], S_[:], ALU.add)
                Y1 = bt(f"{nm}o3")       # o3 dead
                nc.vector.tensor_tensor(Y1[:], y[1][:], y[2][:], ALU.mult)
                Y2 = bt(f"{nm}y1")       # y1 dead
                nc.vector.tensor_tensor(Y2[:], Y1[:], y[4][:], ALU.mult)
                Z_ = bt(f"{nm}y2")       # y2 dead
                nc.vector.tensor_scalar(Z_[:], Y2[:], 2.0, None, ALU.mult)
                det = bt(f"{nm}det")
                nc.vector.tensor_tensor(det[:], D_[:], Z_[:], ALU.add)

                # gc = max(e^3 - 6.75 det^2, GMIN)  (= (p2^3 - 54 det^2)/8)
                gc = ft(f"{nm}gc")
                nc.vector._custom_dve(gclamp, out=gc[:], in0=e[:], in1=det[:],
                                      s0=GMIN, imm2=6.75)
                rec = ft(f"{nm}rec")
                nc.vector.reciprocal_approx_fast(rec[:], gc[:])

                # both Sqrt ops adjacent (one sqrt-table load), then trig set
                p = bt(f"{nm}p0")        # e0 dead
                nc.scalar.activation(p[:], e[:], AF.Sqrt,
                                     bias=peps_ap, scale=p_scale)
                rsg = bt(f"{nm}n11")     # n11 dead
                nc.scalar.activation(rsg[:], rec[:], AF.Sqrt)
                wk = bt(f"{nm}n22")      # n22 dead
                nc.vector.tensor_tensor(wk[:], det[:], rsg[:], ALU.mult)
                nc.scalar.activation(wk[:], wk[:], AF.Arctan, scale=SQRT675)
                nc.scalar.activation(wk[:], wk[:], AF.Sin,
                                     bias=pi3_ap, scale=1.0 / 3.0)   # cs
                return p, wk, tr2

            for _ in range(reps):
                for ch in range(NCH):
                    off = ch * NT
                    _cnt[0] += 1
                    mt = mpool.tile([P, NT], I32, tag="mask",
                                    name=f"mask_{_cnt[0]}")
                    nc.sync.dma_start(
                        mt[:],
                        m_in.rearrange("(p f) -> p f", p=P)[:, off:off + NT])
                    mf = bt("mf")
                    msum = ppool.tile([P, 1], F32, tag="msum")
                    nc.scalar.activation(mf[:], mt[:], AF.Copy,
                                         accum_out=msum[:])
                    nc.vector.tensor_tensor(cacc[:], cacc[:], msum[:], ALU.add)

                    p3i, csi, _ = chain(x_in, off, "i", 3.0)       # 3 p_in
                    p1t, cst, tr2t = chain(t_in, off, "t", 1.0 / 3.0)

                    u = bt("u")
                    nc.vector.tensor_tensor(u[:], p3i[:], csi[:], ALU.mult)
                    v = bt("v")
                    nc.vector.tensor_tensor(v[:], p1t[:], cst[:], ALU.mult)
                    w3 = bt("w3")
                    nc.vector.tensor_tensor(w3[:], u[:], v[:], ALU.add)
                    qt = bt("qt")
                    nc.vector.tensor_scalar(qt[:], tr2t[:], 1.0 / 3.0,
                                            None, ALU.mult)
                    nd = bt("nd")        # q_t - (ani_in + p_t cs_t); |.| later
                    nc.vector.tensor_tensor(nd[:], qt[:], w3[:], ALU.subtract)
                    dm = bt("v")         # v dead
                    nc.vector.tensor_tensor(dm[:], nd[:], mf[:], ALU.mult)

                    asum = ppool.tile([P, 1], F32, tag="asum")
                    nc.scalar.activation(dm[:], dm[:], AF.Abs,
                                         accum_out=asum[:])
                    nc.vector.tensor_tensor(lacc[:], lacc[:], asum[:], ALU.add)

            res = apool.tile([P, 2], F32, tag="res")
            nc.vector.tensor_copy(res[:, 0:1], lacc[:])
            nc.vector.tensor_copy(res[:, 1:2], cacc[:])
            nc.sync.dma_start(out[:], res[:])

    nc.compile()
    return nc


def get_module(reps: int = 1):
    if reps not in _CACHE:
        _CACHE[reps] = _build(reps)
    return _CACHE[reps]


def make_in_maps(input_data, target, mask, gt_mean, gt_std):
    """Shard the full inputs 8 ways along the flattened spatial axis; each
    core gets contiguous per-channel planes [C, B*SH]."""
    xs = np.asarray(input_data).reshape(B, C, HWD)
    ts = np.asarray(target).reshape(B, C, HWD)
    ms = np.asarray(mask).reshape(B, HWD)
    scal = np.zeros((P, 16), np.float32)
    scal[:, 0:6] = np.asarray(gt_std, np.float32).reshape(1, 6)
    scal[:, 6:12] = np.asarray(gt_mean, np.float32).reshape(1, 6)
    scal[:, 12] = PEPS
    scal[:, 13] = PI3
    in_maps = []
    for k in range(N_CORES):
        sl = slice(k * SH, (k + 1) * SH)
        in_maps.append({
            "input_data": np.ascontiguousarray(
                xs[:, :, sl].transpose(1, 0, 2)).reshape(C, BSH),
            "target": np.ascontiguousarray(
                ts[:, :, sl].transpose(1, 0, 2)).reshape(C, BSH),
            "mask": np.ascontiguousarray(ms[:, sl]).reshape(BSH),
            "scal": scal,
        })
    return in_maps


def kernel(input_data, target, mask, gt_mean, gt_std):
    nc = get_module()
    in_maps = make_in_maps(input_data, target, mask, gt_mean, gt_std)
    r = run_bass_kernel_spmd(nc, in_maps, core_ids=list(range(N_CORES)))
    s = 0.0
    n = 0.0
    for i in range(N_CORES):
        o = r.results[i]["out"].astype(np.float64)
        s += o[:, 0].sum()
        n += o[:, 1].sum()
    return np.float32(s / max(n, 1.0))


# revision 10
# speedup vs baseline: 1.1576x; 1.1400x over previous
"""Trainium2 Bass kernel for the ANI (anisotropy) L1 loss - final version.

Math (per voxel, 3x3 symmetric tensor, channels xx,xy,xz,yy,yz,zz):
  y_c = gt_std[c]*x_c + gt_mean[c]
  A = [[y0,y1,y2],[y1,y3,y4],[y2,y4,y5]];  q = tr(A)/3;  C = A - q I
  p2 = ||C||_F^2; p = sqrt(p2/6); det = det(C); r = det/(2 p^3)
  phi = arccos(r)/3
  ani_in  = 3 p cos(phi)        ani_tg = q - p cos(phi)
  loss = sum(|ani_in - ani_tg| * mask) / max(sum(mask), 1)

Device identities (ACT tables lack arccos/cos/rsqrt):
  cos(arccos(r)/3) = sin(pi/3 + arctan(w)/3),  w = r/sqrt(1-r^2)
  w = sqrt(6.75) * det / sqrt(gc),  gc = max(e^3 - 6.75 det^2, GMIN)
  where e = p2/2 (the traceless identity sq00+sq11+sq22 = 2(sq00+sq11+n00*n11)
  removes one square; the factor 2 is folded into constant scales).
  3p_in = sqrt(3 e + eps);  p_tg = sqrt(e/3 + eps).

Mapping: bf16 mid-chain on DVE (tensor ops never write in place - that would
break the 2x bf16 perf mode), affines/squares/sqrt/arctan/sin/abs on ScalarE
(Square/Abs live in every ACT table set; the two Sqrt ops are emitted
adjacently per chain to minimize table-set switches), one fused custom DVE op
for gc, RECIPROCAL_APPROX_FAST for 1/gc. Masked |diff| and mask count reduce
to [128,1] partials via accum_out.

Sharding: pure data-parallel, spatial axis split 8 ways. Each core emits
[128,2] (masked-|diff| sum, mask count) partials; the host reduces the 8x128
pairs and divides - that is the "all-reduce of (masked-sum, mask-count)".
"""

import numpy as np

import concourse.tile as tile
from concourse import bacc, mybir
from concourse.bass_utils import run_bass_kernel_spmd

F32 = mybir.dt.float32
BF16 = mybir.dt.bfloat16
I32 = mybir.dt.int32
ALU = mybir.AluOpType
AF = mybir.ActivationFunctionType

N_CORES = 8
B, C = 4, 6
HWD = 96 * 96 * 96
SH = HWD // N_CORES         # spatial elems per core per (b, c)
BSH = B * SH                # 442368 voxels per core
P = 128
FREE = BSH // P             # 3456
NT = 1728                   # free elems per chunk (2 chunks)
XBUF = 3

SQRT675 = float(np.sqrt(6.75))
GMIN = 1e-30
PEPS = 1e-25
PI3 = float(np.pi / 3.0)

_CACHE = {}
_GCLAMP = None


def _register_gclamp():
    """Register the fused custom DVE op gc = max(in0^3 - imm2*in1^2, s0)."""
    global _GCLAMP
    if _GCLAMP is not None:
        return _GCLAMP
    import concourse.dve_ops as dve_ops
    from concourse.dve_ops import DveOp
    from concourse.dve_spec import Spec, Src0, Src1, C0, C2, maxx, sq, lower, _has_src1
    from concourse.dve_uop import DveOpSpec

    name = "ANI_GCLAMP"
    body = maxx((sq(Src0) * Src0) - (sq(Src1) * C2), C0)

    def ref(in0, in1, c0, c1, c2):
        x = in0.astype(np.float32)
        d = in1.astype(np.float32)
        return np.maximum(x * x * x - d * d * c2, c0)

    spec = Spec(body=body, reference=ref)
    row = dve_ops._CUSTOM_DVE_ROW_BASE + len(dve_ops.OPS)
    tmp = DveOpSpec(name=name, opcode=row, uops=lower(spec, ver="v3"),
                    rd1_en=_has_src1(spec))
    op = DveOp(name, spec, subdim=False, uops_sha={"v3": tmp.sha("v3")})
    dve_ops.OPS.append(op)
    dve_ops.CUSTOM_DVE_SPECS[name] = spec
    dve_ops._SUB_OPCODE_FOR_NAME[name] = row
    _GCLAMP = op
    return op


def _build(reps: int = 1):
    NCH = FREE // NT
    gclamp = _register_gclamp()
    nc = bacc.Bacc("TRN2", target_bir_lowering=False, debug=False,
                   num_devices=N_CORES)
    x_in = nc.dram_tensor("input_data", [C, BSH], F32, kind="ExternalInput")
    t_in = nc.dram_tensor("target", [C, BSH], F32, kind="ExternalInput")
    m_in = nc.dram_tensor("mask", [BSH], I32, kind="ExternalInput")
    sc_in = nc.dram_tensor("scal", [P, 16], F32, kind="ExternalInput")
    out = nc.dram_tensor("out", [P, 2], F32, kind="ExternalOutput")

    with tile.TileContext(nc) as tc:
        with (
            tc.tile_pool(name="const", bufs=1) as cpool,
            tc.tile_pool(name="xio", bufs=XBUF) as xpool,
            tc.tile_pool(name="mio", bufs=1) as mpool,
            tc.tile_pool(name="tmp", bufs=1) as tpool,
            tc.tile_pool(name="acc", bufs=1) as apool,
            tc.tile_pool(name="part", bufs=2) as ppool,
        ):
            scal = cpool.tile([P, 16], F32, tag="scal")
            nc.sync.dma_start(scal[:], sc_in[:])
            lacc = apool.tile([P, 1], F32, tag="lacc")
            cacc = apool.tile([P, 1], F32, tag="cacc")
            nc.vector.memset(lacc[:], 0.0)
            nc.vector.memset(cacc[:], 0.0)

            def s_ap(c):
                return scal[:, c:c + 1]

            def mu_ap(c):
                return scal[:, 6 + c:7 + c]

            peps_ap = scal[:, 12:13]
            pi3_ap = scal[:, 13:14]

            _cnt = [0]

            def bt(tag):
                _cnt[0] += 1
                return tpool.tile([P, NT], BF16, tag=tag,
                                  name=f"b{tag}_{_cnt[0]}")

            def ft(tag):
                _cnt[0] += 1
                return tpool.tile([P, NT], F32, tag=tag,
                                  name=f"f{tag}_{_cnt[0]}")

            def chain(src, off, nm, p_scale):
                """One tensor's full chain. Tag namespace `nm` keeps the two
                tensors' chains independent so the scheduler interleaves them.
                Pool slots are recycled across stages (comments note the dead
                tile being replaced); no DVE op ever writes its own input.
                Returns (p, cs, tr2): p = sqrt(p_scale*e+eps), cs = cos term.
                """
                y = []
                for c in range(C):
                    _cnt[0] += 1
                    xt = xpool.tile([P, NT], F32, tag="x",
                                    name=f"x{nm}{c}_{_cnt[0]}")
                    nc.sync.dma_start(
                        xt[:],
                        src[c].rearrange("(p f) -> p f", p=P)[:, off:off + NT])
                    yt = bt(f"{nm}y{c}")
                    nc.scalar.activation(yt[:], xt[:], AF.Identity,
                                         bias=mu_ap(c), scale=s_ap(c))
                    y.append(yt)

                tr = bt(f"{nm}sa")
                nc.vector.tensor_tensor(tr[:], y[0][:], y[3][:], ALU.add)
                tr2 = bt(f"{nm}tr2")
                nc.vector.tensor_tensor(tr2[:], tr[:], y[5][:], ALU.add)
                q = bt(f"{nm}q")
                nc.vector.tensor_scalar(q[:], tr2[:], 1.0 / 3.0, None, ALU.mult)
                n00 = bt(f"{nm}n00")     # negated deviator diag: n = q - y
                nc.vector.tensor_tensor(n00[:], q[:], y[0][:], ALU.subtract)
                n11 = bt(f"{nm}n11")
                nc.vector.tensor_tensor(n11[:], q[:], y[3][:], ALU.subtract)
                n22 = bt(f"{nm}n22")
                nc.vector.tensor_tensor(n22[:], q[:], y[5][:], ALU.subtract)

                sq00 = bt(f"{nm}y0")     # y0 dead
                nc.scalar.activation(sq00[:], n00[:], AF.Square)
                sq11 = bt(f"{nm}y3")     # y3 dead
                nc.scalar.activation(sq11[:], n11[:], AF.Square)
                o1 = bt(f"{nm}o1")
                nc.scalar.activation(o1[:], y[1][:], AF.Square)
                o2 = bt(f"{nm}o2")
                nc.scalar.activation(o2[:], y[2][:], AF.Square)
                o3 = bt(f"{nm}o3")
                nc.scalar.activation(o3[:], y[4][:], AF.Square)

                pm = bt(f"{nm}q")        # q dead
                nc.vector.tensor_tensor(pm[:], n00[:], n11[:], ALU.mult)

                # e = p2/2 = (sq00+sq11+pm) + (o1+o2+o3), clamped >= 0
                a1 = bt(f"{nm}sa")       # tr dead
                nc.vector.tensor_tensor(a1[:], sq00[:], sq11[:], ALU.add)
                a2 = bt(f"{nm}y5")       # y5 dead
                nc.vector.tensor_tensor(a2[:], a1[:], pm[:], ALU.add)
                b1 = bt(f"{nm}b1")
                nc.vector.tensor_tensor(b1[:], o1[:], o2[:], ALU.add)
                b2 = bt(f"{nm}sa")       # a1 dead
                nc.vector.tensor_tensor(b2[:], b1[:], o3[:], ALU.add)
                e0 = bt(f"{nm}p0")
                nc.vector.tensor_tensor(e0[:], a2[:], b2[:], ALU.add)
                e = bt(f"{nm}p2")        # bf16 rounding can leave e0 < 0 in
                nc.vector.tensor_scalar(  # near-isotropic voxels; sqrt needs >=0
                    e[:], e0[:], 0.0, None, ALU.max)

                # det = n22*(o1 - pm) + n00*o3 + n11*o2 + 2*y1*y2*y4
                K = bt(f"{nm}b1")        # b1 dead
                nc.vector.tensor_tensor(K[:], o1[:], pm[:], ALU.subtract)
                T1 = bt(f"{nm}sa")       # b2 dead
                nc.vector.tensor_tensor(T1[:], n22[:], K[:], ALU.mult)
                A_ = bt(f"{nm}y0")       # sq00 dead
                nc.vector.tensor_tensor(A_[:], n00[:], o3[:], ALU.mult)
                B_ = bt(f"{nm}y3")       # sq11 dead
                nc.vector.tensor_tensor(B_[:], n11[:], o2[:], ALU.mult)
                S_ = bt(f"{nm}o2")       # o2 dead
                nc.vector.tensor_tensor(S_[:], A_[:], B_[:], ALU.add)
                D_ = bt(f"{nm}y5")       # a2 dead
                nc.vector.tensor_tensor(D_[:], T1[:], S_[:], ALU.add)
                Y1 = bt(f"{nm}o3")       # o3 dead
                nc.vector.tensor_tensor(Y1[:], y[1][:], y[2][:], ALU.mult)
                Y2 = bt(f"{nm}y1")       # y1 dead
                nc.vector.tensor_tensor(Y2[:], Y1[:], y[4][:], ALU.mult)
                Z_ = bt(f"{nm}y2")       # y2 dead
                nc.vector.tensor_scalar(Z_[:], Y2[:], 2.0, None, ALU.mult)
                det = bt(f"{nm}det")
                nc.vector.tensor_tensor(det[:], D_[:], Z_[:], ALU.add)

                # gc = max(e^3 - 6.75 det^2, GMIN)  (= (p2^3 - 54 det^2)/8)
                gc = ft(f"{nm}gc")
                nc.vector._custom_dve(gclamp, out=gc[:], in0=e[:], in1=det[:],
                                      s0=GMIN, imm2=6.75)
                rec = ft(f"{nm}rec")
                nc.vector.reciprocal_approx_fast(rec[:], gc[:])

                # both Sqrt ops adjacent (one sqrt-table load), then trig set
                p = bt(f"{nm}p0")        # e0 dead
                nc.scalar.activation(p[:], e[:], AF.Sqrt,
                                     bias=peps_ap, scale=p_scale)
                rsg = bt(f"{nm}n11")     # n11 dead
                nc.scalar.activation(rsg[:], rec[:], AF.Sqrt)
                wk = bt(f"{nm}n22")      # n22 dead
                nc.vector.tensor_tensor(wk[:], det[:], rsg[:], ALU.mult)
                nc.scalar.activation(wk[:], wk[:], AF.Arctan, scale=SQRT675)
                nc.scalar.activation(wk[:], wk[:], AF.Sin,
                                     bias=pi3_ap, scale=1.0 / 3.0)   # cs
                return p, wk, tr2

            for _ in range(reps):
                for ch in range(NCH):
                    off = ch * NT
                    _cnt[0] += 1
                    mt = mpool.tile([P, NT], I32, tag="mask",
                                    name=f"mask_{_cnt[0]}")
                    nc.sync.dma_start(
                        mt[:],
                        m_in.rearrange("(p f) -> p f", p=P)[:, off:off + NT])
                    mf = bt("mf")
                    msum = ppool.tile([P, 1], F32, tag="msum")
                    nc.scalar.activation(mf[:], mt[:], AF.Copy,
                                         accum_out=msum[:])
                    nc.vector.tensor_tensor(cacc[:], cacc[:], msum[:], ALU.add)

                    p3i, csi, _ = chain(x_in, off, "i", 3.0)       # 3 p_in
                    p1t, cst, tr2t = chain(t_in, off, "t", 1.0 / 3.0)

                    u = bt("u")
                    nc.vector.tensor_tensor(u[:], p3i[:], csi[:], ALU.mult)
                    v = bt("v")
                    nc.vector.tensor_tensor(v[:], p1t[:], cst[:], ALU.mult)
                    w3 = bt("w3")
                    nc.vector.tensor_tensor(w3[:], u[:], v[:], ALU.add)
                    qt = bt("qt")
                    nc.vector.tensor_scalar(qt[:], tr2t[:], 1.0 / 3.0,
                                            None, ALU.mult)
                    nd = bt("nd")        # q_t - (ani_in + p_t cs_t); |.| later
                    nc.vector.tensor_tensor(nd[:], qt[:], w3[:], ALU.subtract)
                    dm = bt("v")         # v dead
                    nc.vector.tensor_tensor(dm[:], nd[:], mf[:], ALU.mult)

                    asum = ppool.tile([P, 1], F32, tag="asum")
                    nc.scalar.activation(dm[:], dm[:], AF.Abs,
                                         accum_out=asum[:])
                    nc.vector.tensor_tensor(lacc[:], lacc[:], asum[:], ALU.add)

            res = apool.tile([P, 2], F32, tag="res")
            nc.vector.tensor_copy(res[:, 0:1], lacc[:])
            nc.vector.tensor_copy(res[:, 1:2], cacc[:])
            nc.sync.dma_start(out[:], res[:])

    nc.compile()
    return nc


def get_module(reps: int = 1):
    if reps not in _CACHE:
        _CACHE[reps] = _build(reps)
    return _CACHE[reps]


def make_in_maps(input_data, target, mask, gt_mean, gt_std):
    """Shard the full inputs 8 ways along the flattened spatial axis; each
    core gets contiguous per-channel planes [C, B*SH]."""
    xs = np.asarray(input_data, np.float32).reshape(B, C, HWD)
    ts = np.asarray(target, np.float32).reshape(B, C, HWD)
    ms = np.asarray(mask, np.int32).reshape(B, HWD)
    scal = np.zeros((P, 16), np.float32)
    scal[:, 0:6] = np.asarray(gt_std, np.float32).reshape(1, 6)
    scal[:, 6:12] = np.asarray(gt_mean, np.float32).reshape(1, 6)
    scal[:, 12] = PEPS
    scal[:, 13] = PI3
    in_maps = []
    for k in range(N_CORES):
        sl = slice(k * SH, (k + 1) * SH)
        in_maps.append({
            "input_data": np.ascontiguousarray(
                xs[:, :, sl].transpose(1, 0, 2)).reshape(C, BSH),
            "target": np.ascontiguousarray(
                ts[:, :, sl].transpose(1, 0, 2)).reshape(C, BSH),
            "mask": np.ascontiguousarray(ms[:, sl]).reshape(BSH),
            "scal": scal,
        })
    return in_maps


def kernel(input_data, target, mask, gt_mean, gt_std):
    nc = get_module()
    in_maps = make_in_maps(input_data, target, mask, gt_mean, gt_std)
    r = run_bass_kernel_spmd(nc, in_maps, core_ids=list(range(N_CORES)))
    s = 0.0
    n = 0.0
    for i in range(N_CORES):
        o = r.results[i]["out"].astype(np.float64)
        s += o[:, 0].sum()
        n += o[:, 1].sum()
    return np.float32(s / max(n, 1.0))
